# revision 59
# baseline (speedup 1.0000x reference)
"""Causal self-attention (GQA + RoPE) sharded DP2 x TP4 over 8 trn2 cores.

Core c owns batch c//4 and head-group g=c%4: Q heads {4g..4g+3} plus their
exact GQA KV head g (rep=4), so K/V projections are computed once per
(batch, kv-head) with no duplication. Each core computes its head-shard of
q/k/v projections + rotary + causal attention + a partial o_proj against its
512-column shard of Wo. The host sums 4 partials per batch.

All matmul operands are bf16 (fp32 PSUM accumulation): same PE rate as
float32r but no small-free-size penalty, half the SBUF/DMA footprint.
Measured numerics: rel err ~3.6e-3 vs the 2e-2 gate.

Engine assignment: PE matmuls; ACT exp + yp evac; DVE rope + normalize +
evac halves; GpSimd causal tri-mask. Attention epilogues are emitted one
tile late so the in-order ACT/DVE queues never block the next tile's
critical ops. The rowsum matmul uses an all-ones [128,128] stationary so
the PE broadcasts the softmax denominator across partitions for free; the
whole normalize then stays SBUF/PSUM-local (no DRAM broadcast bounce).
o_proj query-groups interleave into the attention phase two tiles behind
their normalize, filling PE bubbles and spreading the out-DMA; the final
j-tile's normalize runs in 128-column pieces so the last group's
stationaries unblock early (subtile deps).

Layouts (per core):
  xT    [2048, 2048] x[b] transposed (contraction dim on partitions), bf16
  qT/kT [128, 512]   per (head, t-tile), head_dim on partitions
  v_nat [128, 4, 128] natural [t, d] chunks via PE transpose (PV contraction)
  scores kept transposed [tk, tq]: softmax denom via ones-matmul on PE,
  no max subtraction (weights are 0.02-scale, scores are O(1), exp is safe).
Score matmuls are issued one chunk ahead of PV/rowsum so the scalar-engine
exp latency hides behind the next chunk's score matmul.
"""

import sys

try:
    import concourse.bass as bass  # noqa: F401
except ImportError:
    sys.path.insert(0, "/opt/trn_rl_repo")

import math
from contextlib import ExitStack

import ml_dtypes
import numpy as np

import concourse.bass as bass
import concourse.mybir as mybir
import concourse.tile as tile
from concourse import bacc
from concourse.bass_utils import run_bass_kernel_spmd

F32 = mybir.dt.float32
BF16 = mybir.dt.bfloat16

B, T, C = 2, 2048, 2048
N_HEAD, N_KV_HEAD, HD = 16, 4, 128
ROTARY_BASE = 10000
N_CORES = 8
NH = 4  # q heads per core
QSH = NH * HD  # 512 per-core q output dims
SCALE = 1.0 / math.sqrt(HD)

TT = 512  # t-tile (moving-operand free size)
NT = T // TT  # 4
KC = C // 128  # 16 contraction chunks for projections


def _sin_cos_np():
    # mirror reference._sin_cos bit-for-bit (float32 throughout)
    pos = np.arange(T, dtype=np.float32)
    dim = np.arange(HD // 2, dtype=np.float32)
    freq = (np.float32(ROTARY_BASE) ** (dim / np.float32(HD / 2))).astype(np.float32)
    freq = np.concatenate([freq, freq])
    angles = pos[:, None] / freq[None, :]
    return np.sin(angles).astype(np.float32), np.cos(angles).astype(np.float32)


def build_kernel():
    nc = bacc.Bacc()
    xT = nc.dram_tensor("xT", [C, T], BF16, kind="ExternalInput")
    wq = nc.dram_tensor("wq", [C, QSH], BF16, kind="ExternalInput")
    wk = nc.dram_tensor("wk", [C, HD], BF16, kind="ExternalInput")
    wv = nc.dram_tensor("wv", [C, HD], BF16, kind="ExternalInput")
    wo = nc.dram_tensor("wo", [QSH, C], BF16, kind="ExternalInput")
    cosd = nc.dram_tensor("cosd", [HD, T], BF16, kind="ExternalInput")
    sind = nc.dram_tensor("sind", [HD, T], BF16, kind="ExternalInput")  # rot+signed
    trid = nc.dram_tensor("trid", [128, 128], BF16, kind="ExternalInput")
    identd = nc.dram_tensor("identd", [128, 128], BF16, kind="ExternalInput")
    onesd = nc.dram_tensor("onesd", [128, 128], BF16, kind="ExternalInput")
    # bf16 partial output (host upconverts and sums): halves the out-DMA
    out = nc.dram_tensor("out", [T, C], BF16, kind="ExternalOutput")

    with ExitStack() as ctx:
        tc = ctx.enter_context(tile.TileContext(nc))
        consts = ctx.enter_context(tc.tile_pool(name="consts", bufs=1))
        xpool = ctx.enter_context(tc.tile_pool(name="xg", bufs=8))
        qpool = ctx.enter_context(tc.tile_pool(name="qT", bufs=16))
        kpool = ctx.enter_context(tc.tile_pool(name="kT", bufs=4))
        vpool = ctx.enter_context(tc.tile_pool(name="vnat", bufs=4))
        vtpool = ctx.enter_context(tc.tile_pool(name="vt", bufs=2))
        tmppool = ctx.enter_context(tc.tile_pool(name="ropetmp", bufs=3))
        ppool = ctx.enter_context(tc.tile_pool(name="pT", bufs=6))
        ytpool = ctx.enter_context(tc.tile_pool(name="yT", bufs=16))
        ripool = ctx.enter_context(tc.tile_pool(name="rinv", bufs=3))
        outpool = ctx.enter_context(tc.tile_pool(name="osb", bufs=3))

        # PSUM: two tag groups of 4 banks each. "a" carries the rotating
        # streams (q-proj accumulators / score tiles / o_proj accumulators),
        # "b" the longer-lived ones (k/v-proj, v-transpose, PV + rowsum).
        ps = ctx.enter_context(tc.tile_pool(name="ps", bufs=4, space="PSUM"))

        def psa(shape, dtype, name):
            return ps.tile(shape, dtype, tag="a", name=name)

        def psb(shape, dtype, name):
            return ps.tile(shape, dtype, tag="b", name=name)

        # --- resident weights on the ACT queue, grouped 4 contraction
        # chunks per DMA so the queue issues ~24 descriptors, not ~60
        wq_g = [consts.tile([128, 4, QSH], BF16, name=f"wq_{gi}") for gi in range(4)]
        wk_g = [consts.tile([128, 4, HD], BF16, name=f"wk_{gi}") for gi in range(4)]
        wv_g = [consts.tile([128, 4, HD], BF16, name=f"wv_{gi}") for gi in range(4)]
        cos_sb = consts.tile([HD, T], BF16)
        sin_sb = consts.tile([HD, T], BF16)
        tri_sb = consts.tile([128, 128], BF16)
        id_sb = consts.tile([128, 128], BF16)
        # all-ones [128,128] stationary: the rowsum matmul then broadcasts
        # the denominator to every output partition for free
        ones_sb = consts.tile([128, 128], BF16)
        wo_sb = consts.tile([128, NH, C], BF16)

        def wslice(gi):
            return slice(512 * gi, 512 * gi + 512)

        # The scalar (ACT) queue carries weights + rope tables + half of the
        # later x-tiles, emitted in consumption order (need-times in us are
        # against ~117GB/s per queue). The scalar queue starts flowing ~3us
        # before sync, so the very first x chunk rides it too; the first wq
        # group lands chunk-by-chunk so the first matmul waits on 2x128KB.
        xg0 = [
            xpool.tile([128, 4, TT], BF16, tag="xg", name=f"xg_0_{gi}")
            for gi in range(4)
        ]
        nc.scalar.dma_start(
            out=xg0[0][:, 0:1, :],
            in_=xT.ap()[0:128, 0:TT].rearrange("(k p) t -> p k t", p=128),
        )
        nc.scalar.dma_start(
            out=wq_g[0][:, 0:1, :],
            in_=wq.ap()[0:128, :].rearrange("(k p) m -> p k m", p=128),
        )
        nc.scalar.dma_start(
            out=xg0[0][:, 1:2, :],
            in_=xT.ap()[128:256, 0:TT].rearrange("(k p) t -> p k t", p=128),
        )
        nc.scalar.dma_start(
            out=wq_g[0][:, 1:2, :],
            in_=wq.ap()[128:256, :].rearrange("(k p) m -> p k m", p=128),
        )
        nc.scalar.dma_start(
            out=wq_g[0][:, 2:4, :],
            in_=wq.ap()[256:512, :].rearrange("(k p) m -> p k m", p=128),
        )
        nc.scalar.dma_start(
            out=wq_g[1][:, 0:2, :],
            in_=wq.ap()[512:768, :].rearrange("(k p) m -> p k m", p=128),
        )
        nc.scalar.dma_start(
            out=wq_g[1][:, 2:4, :],
            in_=wq.ap()[768:1024, :].rearrange("(k p) m -> p k m", p=128),
        )
        for gi in range(2, 4):
            nc.scalar.dma_start(
                out=wq_g[gi],
                in_=wq.ap()[wslice(gi), :].rearrange("(k p) m -> p k m", p=128),
            )
        cs0 = slice(0, TT)
        nc.scalar.dma_start(out=cos_sb[:, cs0], in_=cosd.ap()[:, cs0])
        nc.scalar.dma_start(out=sin_sb[:, cs0], in_=sind.ap()[:, cs0])
        for gi in range(4):
            nc.scalar.dma_start(
                out=wk_g[gi],
                in_=wk.ap()[wslice(gi), :].rearrange("(k p) m -> p k m", p=128),
            )
            nc.scalar.dma_start(
                out=wv_g[gi],
                in_=wv.ap()[wslice(gi), :].rearrange("(k p) m -> p k m", p=128),
            )
        nc.scalar.dma_start(out=id_sb, in_=identd.ap())
        nc.scalar.dma_start(out=tri_sb, in_=trid.ap())
        nc.scalar.dma_start(out=ones_sb, in_=onesd.ap())

        def load_cos_sin(jt):  # emitted into the jt loop, scalar queue
            cs = slice(TT * jt, TT * jt + TT)
            nc.scalar.dma_start(out=cos_sb[:, cs], in_=cosd.ap()[:, cs])
            nc.scalar.dma_start(out=sin_sb[:, cs], in_=sind.ap()[:, cs])

        def wq_c(kc):  # [128, 128*NH] stationary slice for contraction chunk kc
            return wq_g[kc // 4][:, kc % 4, :]

        def wk_c(kc):
            return wk_g[kc // 4][:, kc % 4, :]

        def wv_c(kc):
            return wv_g[kc // 4][:, kc % 4, :]

        xT_ap = xT.ap()
        out_ap = out.ap()

        def rope_evac(dst, pj, tpos):
            """dst = pj*cos + rotate_half(pj)*sin, psum -> sbuf (bf16).

            sind rows are pre-rotated by 64 and sign-folded on the host.
            """
            cs = cos_sb[:, tpos : tpos + TT]
            sn = sin_sb[:, tpos : tpos + TT]
            tmp = tmppool.tile([128, TT], F32)
            nc.vector.tensor_mul(tmp[0:64], pj[64:128], sn[64:128])
            nc.vector.tensor_mul(tmp[64:128], pj[0:64], sn[0:64])
            nc.vector.tensor_mul(dst, pj, cs)  # last psum read: frees the bank
            nc.vector.tensor_add(dst, dst, tmp)

        # ---------------- projections ----------------
        qT = [
            [
                qpool.tile([128, TT], BF16, tag="qT", name=f"qT_{h}_{j}")
                for j in range(NT)
            ]
            for h in range(NH)
        ]
        kT = [kpool.tile([128, TT], BF16, tag="kT", name=f"kT_{j}") for j in range(NT)]
        v_sb = [
            vpool.tile([128, 4, HD], BF16, tag="v", name=f"v_{j}") for j in range(NT)
        ]

        # V transpose for tile jt, emitted one tile late so the PE never
        # waits on the DVE evac of pv at a tile boundary
        def make_vtrans(jt, pv):
            def emit():
                vt_sb = vtpool.tile([128, TT], BF16, name=f"vt_{jt}")
                nc.vector.tensor_copy(vt_sb, pv)  # frees the pv bank
                vt_ps = psb([128, 4, 128], BF16, f"vtp_{jt}")
                for i in range(4):
                    nc.tensor.transpose(
                        vt_ps[:, i, :], vt_sb[:, 128 * i : 128 * i + 128], id_sb
                    )
                nc.vector.tensor_copy(v_sb[jt], vt_ps)

            return emit

        pending_vt = None
        for jt in range(NT):
            tpos = jt * TT
            if jt == 0:
                xg = xg0
            else:
                xg = [
                    xpool.tile([128, 4, TT], BF16, tag="xg", name=f"xg_{jt}_{gi}")
                    for gi in range(4)
                ]
            if jt >= 1:
                load_cos_sin(jt)
            for gi in range(4):
                src = xT_ap[wslice(gi), tpos : tpos + TT]
                if jt == 0 and gi == 0:
                    # chunks 0,1 already landing via the scalar queue
                    for kk in range(2, 4):
                        nc.sync.dma_start(
                            out=xg[gi][:, kk : kk + 1, :],
                            in_=src[128 * kk : 128 * kk + 128, :].rearrange(
                                "(k p) t -> p k t", p=128
                            ),
                        )
                elif jt == 0:
                    nc.sync.dma_start(
                        out=xg[gi][:, 0:2, :],
                        in_=src[0:256, :].rearrange("(k p) t -> p k t", p=128),
                    )
                    nc.sync.dma_start(
                        out=xg[gi][:, 2:4, :],
                        in_=src[256:512, :].rearrange("(k p) t -> p k t", p=128),
                    )
                else:
                    # split each later x-tile over the two hw DGE queues
                    # (SP + ACT): one queue (~120GB/s) can't keep 2MB/tile
                    # ahead of the PE, and jt0 must own all of SP early
                    eng = nc.sync if gi < 2 else nc.scalar
                    eng.dma_start(
                        out=xg[gi], in_=src.rearrange("(k p) t -> p k t", p=128)
                    )

            def xc(kc):
                return xg[kc // 4][:, kc % 4, :]

            pq = [psa([128, TT], F32, f"pq_{jt}_{h}") for h in range(NH)]
            for kc in range(KC):
                st, sp = (kc == 0), (kc == KC - 1)
                for h in range(NH):
                    nc.tensor.matmul(
                        pq[h],
                        wq_c(kc)[:, 128 * h : 128 * h + 128],
                        xc(kc),
                        start=st,
                        stop=sp,
                    )
            if jt == NT - 1:
                nc.scalar.dma_start(
                    out=wo_sb, in_=wo.ap().rearrange("(h p) n -> p h n", p=128)
                )
            pk = psb([128, TT], F32, f"pk_{jt}")
            pv = psb([128, TT], F32, f"pv_{jt}")
            for kc in range(KC):
                st, sp = (kc == 0), (kc == KC - 1)
                nc.tensor.matmul(pk, wk_c(kc), xc(kc), start=st, stop=sp)
                nc.tensor.matmul(pv, wv_c(kc), xc(kc), start=st, stop=sp)
            if pending_vt is not None:
                pending_vt()
            for h in range(NH):
                rope_evac(qT[h][jt], pq[h], tpos)
            rope_evac(kT[jt], pk, tpos)
            pending_vt = make_vtrans(jt, pv)
        pending_vt()

        # ---------------- attention ----------------
        yT = [
            [
                ytpool.tile([128, TT], BF16, tag="yT", name=f"yT_{h}_{j}")
                for j in range(NT)
            ]
            for h in range(NH)
        ]
        def make_evac(j, h, yp):
            def emit():
                # ACT copy (Copy shares the Exp table set, no reload): frees
                # the PV bank at ACT-queue speed
                nc.scalar.copy(yT[h][j], yp)

            return emit

        def make_norm(j, h, rp):
            def emit():
                # rp holds the rowsum broadcast across partitions (all-ones
                # stationary): normalize on the DVE with no DRAM bounce. The
                # reciprocal costs ~6.5ns/column (~3.3us) so it is emitted
                # two tiles late — younger bank-freeing ops never queue
                # behind it on the in-order DVE queue.
                rinv = ripool.tile([128, TT], F32, tag="ri", name=f"ri_{h}_{j}")
                if j == NT - 1:
                    # piecewise: o_proj tiles read 128-col slices of yT, so
                    # the first slice's normalize unblocks the final o_proj
                    # group ~4us earlier (reciprocal is ~6.5ns/column); for
                    # earlier tiles the monolithic form wins — it frees the
                    # rowsum bank in one shot, which the small-tile stretch
                    # needs more than early unblocking
                    for m in range(4):
                        cs = slice(128 * m, 128 * m + 128)
                        nc.vector.reciprocal(rinv[:, cs], rp[:, cs])
                        nc.vector.tensor_mul(
                            yT[h][j][:, cs], yT[h][j][:, cs], rinv[:, cs]
                        )
                else:
                    nc.vector.reciprocal(rinv, rp)  # frees the rowsum bank
                    nc.vector.tensor_mul(yT[h][j], yT[h][j], rinv)

            return emit

        # o_proj for query group j (rows 512j..512j+512), interleaved into
        # the attention phase two tiles after norm(j) so the normalized yT
        # slices are ready; spreads the out-DMA and fills PE bubbles
        def emit_oproj_group(j):
            for ts_ in range(4 * j, 4 * j + 4):
                row = 128 * ts_
                osb = outpool.tile([128, C], BF16, tag="osb", name=f"osb_{ts_}")
                for n in range(C // TT):
                    op = psa([128, TT], F32, f"op_{ts_}_{n}")
                    for hh in range(NH):
                        nc.tensor.matmul(
                            op,
                            yT[hh][j][:, 128 * (ts_ % 4) : 128 * (ts_ % 4) + 128],
                            wo_sb[:, hh, TT * n : TT * n + TT],
                            start=(hh == 0),
                            stop=(hh == NH - 1),
                        )
                    nc.scalar.copy(osb[:, TT * n : TT * n + 256], op[:, 0:256])
                    nc.vector.tensor_copy(
                        osb[:, TT * n + 256 : TT * n + TT], op[:, 256:TT]
                    )
                    nc.sync.dma_start(
                        out=out_ap[row : row + 128, TT * n : TT * n + TT],
                        in_=osb[:, TT * n : TT * n + TT],
                    )

        pending = []  # deferred epilogue/norm emissions, one tile late
        # j=1 first, then j=0 with an o_proj group interleaved: the tiny
        # j=0 tiles can't hide the per-tile reciprocal on the DVE, so give
        # that stretch o_proj matmuls to chew on. Groups are emitted two
        # tiles after their last norm.
        J_ORDER = [1, 0, 2, 3]
        GROUP_AT = {(0, 1): 1, (2, 2): 0, (3, 2): 2}
        for j in J_ORDER:
            if j == 0:
                chunks = [(m, 128 * m) for m in (0, 1, 2, 3)]
            else:
                # diagonal chunks last: their exp -> tri-mask chain gets the
                # whole tile of slack before the PV matmul needs the result
                chunks = [(c, 0) for c in range(4 * j)]
                chunks += [(4 * j + m, 128 * m) for m in (0, 1, 2, 3)]
            nch = len(chunks)
            for h in range(NH):
                yp = psb([128, TT], F32, f"yp_{h}_{j}")
                rp = psb([128, TT], F32, f"rp_{h}_{j}")
                sT = [None] * nch
                pT = [None] * nch

                def issue_score(idx):
                    cch, off = chunks[idx]
                    sT[idx] = psa([128, TT], F32, f"sT_{h}_{j}_{idx}")
                    nc.tensor.matmul(
                        sT[idx][:, off:],
                        kT[cch // 4][:, 128 * (cch % 4) : 128 * (cch % 4) + 128],
                        qT[h][j][:, off:],
                        start=True,
                        stop=True,
                    )

                def issue_exp(idx):
                    cch, off = chunks[idx]
                    pT[idx] = ppool.tile(
                        [128, TT], BF16, tag="p", name=f"pT_{h}_{j}_{idx}"
                    )
                    nc.scalar.activation(
                        out=pT[idx][:, off:],
                        in_=sT[idx][:, off:],
                        func=mybir.ActivationFunctionType.Exp,
                        scale=SCALE,
                    )
                    if cch >= 4 * j:  # diagonal block: causal triangle
                        nc.gpsimd.tensor_mul(
                            pT[idx][:, off : off + 128],
                            pT[idx][:, off : off + 128],
                            tri_sb,
                        )

                issue_score(0)
                issue_exp(0)
                for idx in range(nch):
                    cch, off = chunks[idx]
                    if idx + 1 < nch:
                        issue_score(idx + 1)
                        issue_exp(idx + 1)
                    nc.tensor.matmul(
                        yp[:, off:],
                        v_sb[cch // 4][:, cch % 4, :],
                        pT[idx][:, off:],
                        start=(idx == 0),
                        stop=(idx == nch - 1),
                    )
                    nc.tensor.matmul(
                        rp[:, off:],
                        ones_sb,
                        pT[idx][:, off:],
                        start=(idx == 0),
                        stop=(idx == nch - 1),
                    )
                # flush the previous tile's epilogue/norm now that this
                # tile's critical ops are ahead of them in the queues
                for fn in pending:
                    fn()
                pending = [make_evac(j, h, yp), make_norm(j, h, rp)]
                if (j, h) in GROUP_AT:
                    emit_oproj_group(GROUP_AT[(j, h)])
        for fn in pending:
            fn()
        emit_oproj_group(NT - 1)

    nc.finalize()
    return nc


_NC_CACHE = None
TRACE = False
LAST_RESULTS = None


def _get_nc():
    global _NC_CACHE
    if _NC_CACHE is None:
        _NC_CACHE = build_kernel()
    return _NC_CACHE


def kernel(x, Wq, Wk, Wv, Wo):
    bf16 = ml_dtypes.bfloat16
    x = np.asarray(x, dtype=np.float32)
    Wq = np.asarray(Wq, dtype=np.float32)
    Wk = np.asarray(Wk, dtype=np.float32)
    Wv = np.asarray(Wv, dtype=np.float32)
    Wo = np.asarray(Wo, dtype=np.float32)

    sin_, cos_ = _sin_cos_np()  # [T, 128]
    cosd = np.ascontiguousarray(cos_.T).astype(bf16)
    sinT = np.ascontiguousarray(sin_.T)
    # row-rotated by 64 and sign-folded: output rows 0:64 read input rows
    # 64:128 (value -sin), output rows 64:128 read input rows 0:64 (+sin)
    sind = np.empty_like(sinT)
    sind[64:128] = -sinT[0:64]
    sind[0:64] = sinT[64:128]
    sind = sind.astype(bf16)
    trid = np.triu(np.ones((128, 128), dtype=np.float32)).astype(bf16)
    identd = np.eye(128, dtype=np.float32).astype(bf16)
    onesd = np.ones((128, 128), dtype=np.float32).astype(bf16)

    xTb = [np.ascontiguousarray(x[b].T).astype(bf16) for b in range(B)]

    core_ids = list(range(N_CORES))
    in_maps = []
    for c in core_ids:
        b, g = c // 4, c % 4
        in_maps.append(
            {
                "xT": xTb[b],
                "wq": np.ascontiguousarray(Wq[QSH * g : QSH * (g + 1)].T).astype(bf16),
                "wk": np.ascontiguousarray(Wk[HD * g : HD * (g + 1)].T).astype(bf16),
                "wv": np.ascontiguousarray(Wv[HD * g : HD * (g + 1)].T).astype(bf16),
                "wo": np.ascontiguousarray(Wo[:, QSH * g : QSH * (g + 1)].T).astype(
                    bf16
                ),
                "cosd": cosd,
                "sind": sind,
                "trid": trid,
                "identd": identd,
                "onesd": onesd,
            }
        )
    global LAST_RESULTS
    res = run_bass_kernel_spmd(_get_nc(), in_maps, core_ids, trace=TRACE)
    LAST_RESULTS = res
    out = np.empty((B, T, C), np.float32)
    for b in range(B):
        tot = res.results[4 * b]["out"].astype(np.float32)
        for g in range(1, 4):
            tot = tot + res.results[4 * b + g]["out"].astype(np.float32)
        out[b] = tot
    return out


# revision 60
# speedup vs baseline: 1.0049x; 1.0049x over previous
"""Causal self-attention (GQA + RoPE) sharded DP2 x TP4 over 8 trn2 cores.

Core c owns batch c//4 and head-group g=c%4: Q heads {4g..4g+3} plus their
exact GQA KV head g (rep=4), so K/V projections are computed once per
(batch, kv-head) with no duplication. Each core computes its head-shard of
q/k/v projections + rotary + causal attention + a partial o_proj against its
512-column shard of Wo. The host sums 4 partials per batch.

All matmul operands are bf16 (fp32 PSUM accumulation): same PE rate as
float32r but no small-free-size penalty, half the SBUF/DMA footprint.
Measured numerics: rel err ~3.6e-3 vs the 2e-2 gate.

Engine assignment: PE matmuls; ACT exp + yp evac; DVE rope + normalize +
evac halves; GpSimd causal tri-mask. Attention epilogues are emitted one
tile late so the in-order ACT/DVE queues never block the next tile's
critical ops. The rowsum matmul uses an all-ones [128,128] stationary so
the PE broadcasts the softmax denominator across partitions for free; the
whole normalize then stays SBUF/PSUM-local (no DRAM broadcast bounce).
o_proj query-groups interleave into the attention phase two tiles behind
their normalize, filling PE bubbles and spreading the out-DMA; the final
j-tile's normalize runs in 128-column pieces so the last group's
stationaries unblock early (subtile deps).

Layouts (per core):
  xT    [2048, 2048] x[b] transposed (contraction dim on partitions), bf16
  qT/kT [128, 512]   per (head, t-tile), head_dim on partitions
  v_nat [128, 4, 128] natural [t, d] chunks via PE transpose (PV contraction)
  scores kept transposed [tk, tq]: softmax denom via ones-matmul on PE,
  no max subtraction (weights are 0.02-scale, scores are O(1), exp is safe).
Score matmuls are issued one chunk ahead of PV/rowsum so the scalar-engine
exp latency hides behind the next chunk's score matmul.
"""

import sys

try:
    import concourse.bass as bass  # noqa: F401
except ImportError:
    sys.path.insert(0, "/opt/trn_rl_repo")

import math
from contextlib import ExitStack

import ml_dtypes
import numpy as np

import concourse.bass as bass
import concourse.mybir as mybir
import concourse.tile as tile
from concourse import bacc
from concourse.bass_utils import run_bass_kernel_spmd

F32 = mybir.dt.float32
BF16 = mybir.dt.bfloat16

B, T, C = 2, 2048, 2048
N_HEAD, N_KV_HEAD, HD = 16, 4, 128
ROTARY_BASE = 10000
N_CORES = 8
NH = 4  # q heads per core
QSH = NH * HD  # 512 per-core q output dims
SCALE = 1.0 / math.sqrt(HD)

TT = 512  # t-tile (moving-operand free size)
NT = T // TT  # 4
KC = C // 128  # 16 contraction chunks for projections


def _sin_cos_np():
    # mirror reference._sin_cos bit-for-bit (float32 throughout)
    pos = np.arange(T, dtype=np.float32)
    dim = np.arange(HD // 2, dtype=np.float32)
    freq = (np.float32(ROTARY_BASE) ** (dim / np.float32(HD / 2))).astype(np.float32)
    freq = np.concatenate([freq, freq])
    angles = pos[:, None] / freq[None, :]
    return np.sin(angles).astype(np.float32), np.cos(angles).astype(np.float32)


def build_kernel():
    nc = bacc.Bacc()
    xT = nc.dram_tensor("xT", [C, T], BF16, kind="ExternalInput")
    wq = nc.dram_tensor("wq", [C, QSH], BF16, kind="ExternalInput")
    wk = nc.dram_tensor("wk", [C, HD], BF16, kind="ExternalInput")
    wv = nc.dram_tensor("wv", [C, HD], BF16, kind="ExternalInput")
    wo = nc.dram_tensor("wo", [QSH, C], BF16, kind="ExternalInput")
    cosd = nc.dram_tensor("cosd", [HD, T], BF16, kind="ExternalInput")
    sind = nc.dram_tensor("sind", [HD, T], BF16, kind="ExternalInput")  # rot+signed
    trid = nc.dram_tensor("trid", [128, 128], BF16, kind="ExternalInput")
    identd = nc.dram_tensor("identd", [128, 128], BF16, kind="ExternalInput")
    onesd = nc.dram_tensor("onesd", [128, 128], BF16, kind="ExternalInput")
    # bf16 partial output (host upconverts and sums): halves the out-DMA
    out = nc.dram_tensor("out", [T, C], BF16, kind="ExternalOutput")

    with ExitStack() as ctx:
        tc = ctx.enter_context(tile.TileContext(nc))
        consts = ctx.enter_context(tc.tile_pool(name="consts", bufs=1))
        xpool = ctx.enter_context(tc.tile_pool(name="xg", bufs=8))
        qpool = ctx.enter_context(tc.tile_pool(name="qT", bufs=16))
        kpool = ctx.enter_context(tc.tile_pool(name="kT", bufs=4))
        vpool = ctx.enter_context(tc.tile_pool(name="vnat", bufs=4))
        vtpool = ctx.enter_context(tc.tile_pool(name="vt", bufs=2))
        tmppool = ctx.enter_context(tc.tile_pool(name="ropetmp", bufs=3))
        ppool = ctx.enter_context(tc.tile_pool(name="pT", bufs=6))
        ytpool = ctx.enter_context(tc.tile_pool(name="yT", bufs=16))
        ripool = ctx.enter_context(tc.tile_pool(name="rinv", bufs=3))
        outpool = ctx.enter_context(tc.tile_pool(name="osb", bufs=4))

        # PSUM: two tag groups of 4 banks each. "a" carries the rotating
        # streams (q-proj accumulators / score tiles / o_proj accumulators),
        # "b" the longer-lived ones (k/v-proj, v-transpose, PV + rowsum).
        ps = ctx.enter_context(tc.tile_pool(name="ps", bufs=4, space="PSUM"))

        def psa(shape, dtype, name):
            return ps.tile(shape, dtype, tag="a", name=name)

        def psb(shape, dtype, name):
            return ps.tile(shape, dtype, tag="b", name=name)

        # --- resident weights on the ACT queue, grouped 4 contraction
        # chunks per DMA so the queue issues ~24 descriptors, not ~60
        wq_g = [consts.tile([128, 4, QSH], BF16, name=f"wq_{gi}") for gi in range(4)]
        wk_g = [consts.tile([128, 4, HD], BF16, name=f"wk_{gi}") for gi in range(4)]
        wv_g = [consts.tile([128, 4, HD], BF16, name=f"wv_{gi}") for gi in range(4)]
        cos_sb = consts.tile([HD, T], BF16)
        sin_sb = consts.tile([HD, T], BF16)
        tri_sb = consts.tile([128, 128], BF16)
        id_sb = consts.tile([128, 128], BF16)
        # all-ones [128,128] stationary: the rowsum matmul then broadcasts
        # the denominator to every output partition for free
        ones_sb = consts.tile([128, 128], BF16)
        wo_sb = consts.tile([128, NH, C], BF16)

        def wslice(gi):
            return slice(512 * gi, 512 * gi + 512)

        # The scalar (ACT) queue carries weights + rope tables + half of the
        # later x-tiles, emitted in consumption order (need-times in us are
        # against ~117GB/s per queue). The scalar queue starts flowing ~3us
        # before sync, so the very first x chunk rides it too; the first wq
        # group lands chunk-by-chunk so the first matmul waits on 2x128KB.
        xg0 = [
            xpool.tile([128, 4, TT], BF16, tag="xg", name=f"xg_0_{gi}")
            for gi in range(4)
        ]
        nc.scalar.dma_start(
            out=xg0[0][:, 0:1, :],
            in_=xT.ap()[0:128, 0:TT].rearrange("(k p) t -> p k t", p=128),
        )
        nc.scalar.dma_start(
            out=wq_g[0][:, 0:1, :],
            in_=wq.ap()[0:128, :].rearrange("(k p) m -> p k m", p=128),
        )
        nc.scalar.dma_start(
            out=xg0[0][:, 1:2, :],
            in_=xT.ap()[128:256, 0:TT].rearrange("(k p) t -> p k t", p=128),
        )
        nc.scalar.dma_start(
            out=wq_g[0][:, 1:2, :],
            in_=wq.ap()[128:256, :].rearrange("(k p) m -> p k m", p=128),
        )
        nc.scalar.dma_start(
            out=wq_g[0][:, 2:4, :],
            in_=wq.ap()[256:512, :].rearrange("(k p) m -> p k m", p=128),
        )
        nc.scalar.dma_start(
            out=wq_g[1][:, 0:2, :],
            in_=wq.ap()[512:768, :].rearrange("(k p) m -> p k m", p=128),
        )
        nc.scalar.dma_start(
            out=wq_g[1][:, 2:4, :],
            in_=wq.ap()[768:1024, :].rearrange("(k p) m -> p k m", p=128),
        )
        for gi in range(2, 4):
            nc.scalar.dma_start(
                out=wq_g[gi],
                in_=wq.ap()[wslice(gi), :].rearrange("(k p) m -> p k m", p=128),
            )
        cs0 = slice(0, TT)
        nc.scalar.dma_start(out=cos_sb[:, cs0], in_=cosd.ap()[:, cs0])
        nc.scalar.dma_start(out=sin_sb[:, cs0], in_=sind.ap()[:, cs0])
        for gi in range(4):
            nc.scalar.dma_start(
                out=wk_g[gi],
                in_=wk.ap()[wslice(gi), :].rearrange("(k p) m -> p k m", p=128),
            )
            nc.scalar.dma_start(
                out=wv_g[gi],
                in_=wv.ap()[wslice(gi), :].rearrange("(k p) m -> p k m", p=128),
            )
        nc.scalar.dma_start(out=id_sb, in_=identd.ap())
        nc.scalar.dma_start(out=tri_sb, in_=trid.ap())
        nc.scalar.dma_start(out=ones_sb, in_=onesd.ap())

        def load_cos_sin(jt):  # emitted into the jt loop, scalar queue
            cs = slice(TT * jt, TT * jt + TT)
            nc.scalar.dma_start(out=cos_sb[:, cs], in_=cosd.ap()[:, cs])
            nc.scalar.dma_start(out=sin_sb[:, cs], in_=sind.ap()[:, cs])

        def wq_c(kc):  # [128, 128*NH] stationary slice for contraction chunk kc
            return wq_g[kc // 4][:, kc % 4, :]

        def wk_c(kc):
            return wk_g[kc // 4][:, kc % 4, :]

        def wv_c(kc):
            return wv_g[kc // 4][:, kc % 4, :]

        xT_ap = xT.ap()
        out_ap = out.ap()

        def rope_evac(dst, pj, tpos):
            """dst = pj*cos + rotate_half(pj)*sin, psum -> sbuf (bf16).

            sind rows are pre-rotated by 64 and sign-folded on the host.
            """
            cs = cos_sb[:, tpos : tpos + TT]
            sn = sin_sb[:, tpos : tpos + TT]
            tmp = tmppool.tile([128, TT], F32)
            nc.vector.tensor_mul(tmp[0:64], pj[64:128], sn[64:128])
            nc.vector.tensor_mul(tmp[64:128], pj[0:64], sn[0:64])
            nc.vector.tensor_mul(dst, pj, cs)  # last psum read: frees the bank
            nc.vector.tensor_add(dst, dst, tmp)

        # ---------------- projections ----------------
        qT = [
            [
                qpool.tile([128, TT], BF16, tag="qT", name=f"qT_{h}_{j}")
                for j in range(NT)
            ]
            for h in range(NH)
        ]
        kT = [kpool.tile([128, TT], BF16, tag="kT", name=f"kT_{j}") for j in range(NT)]
        v_sb = [
            vpool.tile([128, 4, HD], BF16, tag="v", name=f"v_{j}") for j in range(NT)
        ]

        # V transpose for tile jt, emitted one tile late so the PE never
        # waits on the DVE evac of pv at a tile boundary
        def make_vtrans(jt, pv):
            def emit():
                vt_sb = vtpool.tile([128, TT], BF16, name=f"vt_{jt}")
                nc.vector.tensor_copy(vt_sb, pv)  # frees the pv bank
                vt_ps = psb([128, 4, 128], BF16, f"vtp_{jt}")
                for i in range(4):
                    nc.tensor.transpose(
                        vt_ps[:, i, :], vt_sb[:, 128 * i : 128 * i + 128], id_sb
                    )
                nc.vector.tensor_copy(v_sb[jt], vt_ps)

            return emit

        pending_vt = None
        for jt in range(NT):
            tpos = jt * TT
            if jt == 0:
                xg = xg0
            else:
                xg = [
                    xpool.tile([128, 4, TT], BF16, tag="xg", name=f"xg_{jt}_{gi}")
                    for gi in range(4)
                ]
            if jt >= 1:
                load_cos_sin(jt)
            for gi in range(4):
                src = xT_ap[wslice(gi), tpos : tpos + TT]
                if jt == 0 and gi == 0:
                    # chunks 0,1 already landing via the scalar queue
                    for kk in range(2, 4):
                        nc.sync.dma_start(
                            out=xg[gi][:, kk : kk + 1, :],
                            in_=src[128 * kk : 128 * kk + 128, :].rearrange(
                                "(k p) t -> p k t", p=128
                            ),
                        )
                elif jt == 0:
                    nc.sync.dma_start(
                        out=xg[gi][:, 0:2, :],
                        in_=src[0:256, :].rearrange("(k p) t -> p k t", p=128),
                    )
                    nc.sync.dma_start(
                        out=xg[gi][:, 2:4, :],
                        in_=src[256:512, :].rearrange("(k p) t -> p k t", p=128),
                    )
                else:
                    # split each later x-tile over the two hw DGE queues
                    # (SP + ACT): one queue (~120GB/s) can't keep 2MB/tile
                    # ahead of the PE, and jt0 must own all of SP early
                    eng = nc.sync if gi < 2 else nc.scalar
                    eng.dma_start(
                        out=xg[gi], in_=src.rearrange("(k p) t -> p k t", p=128)
                    )

            def xc(kc):
                return xg[kc // 4][:, kc % 4, :]

            pq = [psa([128, TT], F32, f"pq_{jt}_{h}") for h in range(NH)]
            for kc in range(KC):
                st, sp = (kc == 0), (kc == KC - 1)
                for h in range(NH):
                    nc.tensor.matmul(
                        pq[h],
                        wq_c(kc)[:, 128 * h : 128 * h + 128],
                        xc(kc),
                        start=st,
                        stop=sp,
                    )
            if jt == NT - 1:
                nc.scalar.dma_start(
                    out=wo_sb, in_=wo.ap().rearrange("(h p) n -> p h n", p=128)
                )
            pk = psb([128, TT], F32, f"pk_{jt}")
            pv = psb([128, TT], F32, f"pv_{jt}")
            for kc in range(KC):
                st, sp = (kc == 0), (kc == KC - 1)
                nc.tensor.matmul(pk, wk_c(kc), xc(kc), start=st, stop=sp)
                nc.tensor.matmul(pv, wv_c(kc), xc(kc), start=st, stop=sp)
            if pending_vt is not None:
                pending_vt()
            for h in range(NH):
                rope_evac(qT[h][jt], pq[h], tpos)
            rope_evac(kT[jt], pk, tpos)
            pending_vt = make_vtrans(jt, pv)
        pending_vt()

        # ---------------- attention ----------------
        yT = [
            [
                ytpool.tile([128, TT], BF16, tag="yT", name=f"yT_{h}_{j}")
                for j in range(NT)
            ]
            for h in range(NH)
        ]
        def make_evac(j, h, yp):
            def emit():
                # ACT copy (Copy shares the Exp table set, no reload): frees
                # the PV bank at ACT-queue speed
                nc.scalar.copy(yT[h][j], yp)

            return emit

        def make_norm(j, h, rp):
            def emit():
                # rp holds the rowsum broadcast across partitions (all-ones
                # stationary): normalize on the DVE with no DRAM bounce. The
                # reciprocal costs ~6.5ns/column (~3.3us) so it is emitted
                # two tiles late — younger bank-freeing ops never queue
                # behind it on the in-order DVE queue.
                rinv = ripool.tile([128, TT], F32, tag="ri", name=f"ri_{h}_{j}")
                if j == NT - 1:
                    # piecewise: o_proj tiles read 128-col slices of yT, so
                    # the first slice's normalize unblocks the final o_proj
                    # group ~4us earlier (reciprocal is ~6.5ns/column); for
                    # earlier tiles the monolithic form wins — it frees the
                    # rowsum bank in one shot, which the small-tile stretch
                    # needs more than early unblocking
                    for m in range(4):
                        cs = slice(128 * m, 128 * m + 128)
                        nc.vector.reciprocal(rinv[:, cs], rp[:, cs])
                        nc.vector.tensor_mul(
                            yT[h][j][:, cs], yT[h][j][:, cs], rinv[:, cs]
                        )
                else:
                    nc.vector.reciprocal(rinv, rp)  # frees the rowsum bank
                    nc.vector.tensor_mul(yT[h][j], yT[h][j], rinv)

            return emit

        # o_proj for query group j (rows 512j..512j+512), interleaved into
        # the attention phase two tiles after norm(j) so the normalized yT
        # slices are ready; spreads the out-DMA and fills PE bubbles
        def emit_oproj_group(j):
            for ts_ in range(4 * j, 4 * j + 4):
                row = 128 * ts_
                osb = outpool.tile([128, C], BF16, tag="osb", name=f"osb_{ts_}")
                for n in range(C // TT):
                    op = psa([128, TT], F32, f"op_{ts_}_{n}")
                    for hh in range(NH):
                        nc.tensor.matmul(
                            op,
                            yT[hh][j][:, 128 * (ts_ % 4) : 128 * (ts_ % 4) + 128],
                            wo_sb[:, hh, TT * n : TT * n + TT],
                            start=(hh == 0),
                            stop=(hh == NH - 1),
                        )
                    nc.scalar.copy(osb[:, TT * n : TT * n + 256], op[:, 0:256])
                    nc.vector.tensor_copy(
                        osb[:, TT * n + 256 : TT * n + TT], op[:, 256:TT]
                    )
                    nc.sync.dma_start(
                        out=out_ap[row : row + 128, TT * n : TT * n + TT],
                        in_=osb[:, TT * n : TT * n + TT],
                    )

        pending = []  # deferred epilogue/norm emissions, one tile late
        # j=1 first, then j=0 with an o_proj group interleaved: the tiny
        # j=0 tiles can't hide the per-tile reciprocal on the DVE, so give
        # that stretch o_proj matmuls to chew on. Groups are emitted two
        # tiles after their last norm.
        J_ORDER = [1, 0, 2, 3]
        GROUP_AT = {(0, 1): 1, (2, 2): 0, (3, 2): 2}
        for j in J_ORDER:
            if j == 0:
                chunks = [(m, 128 * m) for m in (0, 1, 2, 3)]
            else:
                # diagonal chunks last: their exp -> tri-mask chain gets the
                # whole tile of slack before the PV matmul needs the result
                chunks = [(c, 0) for c in range(4 * j)]
                chunks += [(4 * j + m, 128 * m) for m in (0, 1, 2, 3)]
            nch = len(chunks)
            for h in range(NH):
                yp = psb([128, TT], F32, f"yp_{h}_{j}")
                rp = psb([128, TT], F32, f"rp_{h}_{j}")
                sT = [None] * nch
                pT = [None] * nch

                def issue_score(idx):
                    cch, off = chunks[idx]
                    sT[idx] = psa([128, TT], F32, f"sT_{h}_{j}_{idx}")
                    nc.tensor.matmul(
                        sT[idx][:, off:],
                        kT[cch // 4][:, 128 * (cch % 4) : 128 * (cch % 4) + 128],
                        qT[h][j][:, off:],
                        start=True,
                        stop=True,
                    )

                def issue_exp(idx):
                    cch, off = chunks[idx]
                    pT[idx] = ppool.tile(
                        [128, TT], BF16, tag="p", name=f"pT_{h}_{j}_{idx}"
                    )
                    nc.scalar.activation(
                        out=pT[idx][:, off:],
                        in_=sT[idx][:, off:],
                        func=mybir.ActivationFunctionType.Exp,
                        scale=SCALE,
                    )
                    if cch >= 4 * j:  # diagonal block: causal triangle
                        nc.gpsimd.tensor_mul(
                            pT[idx][:, off : off + 128],
                            pT[idx][:, off : off + 128],
                            tri_sb,
                        )

                issue_score(0)
                issue_exp(0)
                for idx in range(nch):
                    cch, off = chunks[idx]
                    if idx + 1 < nch:
                        issue_score(idx + 1)
                        issue_exp(idx + 1)
                    nc.tensor.matmul(
                        yp[:, off:],
                        v_sb[cch // 4][:, cch % 4, :],
                        pT[idx][:, off:],
                        start=(idx == 0),
                        stop=(idx == nch - 1),
                    )
                    nc.tensor.matmul(
                        rp[:, off:],
                        ones_sb,
                        pT[idx][:, off:],
                        start=(idx == 0),
                        stop=(idx == nch - 1),
                    )
                # flush the previous tile's epilogue/norm now that this
                # tile's critical ops are ahead of them in the queues
                for fn in pending:
                    fn()
                pending = [make_evac(j, h, yp), make_norm(j, h, rp)]
                if (j, h) in GROUP_AT:
                    emit_oproj_group(GROUP_AT[(j, h)])
        for fn in pending:
            fn()
        emit_oproj_group(NT - 1)

    nc.finalize()
    return nc


_NC_CACHE = None
TRACE = False
LAST_RESULTS = None


def _get_nc():
    global _NC_CACHE
    if _NC_CACHE is None:
        _NC_CACHE = build_kernel()
    return _NC_CACHE


def kernel(x, Wq, Wk, Wv, Wo):
    bf16 = ml_dtypes.bfloat16
    x = np.asarray(x, dtype=np.float32)
    Wq = np.asarray(Wq, dtype=np.float32)
    Wk = np.asarray(Wk, dtype=np.float32)
    Wv = np.asarray(Wv, dtype=np.float32)
    Wo = np.asarray(Wo, dtype=np.float32)

    sin_, cos_ = _sin_cos_np()  # [T, 128]
    cosd = np.ascontiguousarray(cos_.T).astype(bf16)
    sinT = np.ascontiguousarray(sin_.T)
    # row-rotated by 64 and sign-folded: output rows 0:64 read input rows
    # 64:128 (value -sin), output rows 64:128 read input rows 0:64 (+sin)
    sind = np.empty_like(sinT)
    sind[64:128] = -sinT[0:64]
    sind[0:64] = sinT[64:128]
    sind = sind.astype(bf16)
    trid = np.triu(np.ones((128, 128), dtype=np.float32)).astype(bf16)
    identd = np.eye(128, dtype=np.float32).astype(bf16)
    onesd = np.ones((128, 128), dtype=np.float32).astype(bf16)

    xTb = [np.ascontiguousarray(x[b].T).astype(bf16) for b in range(B)]

    core_ids = list(range(N_CORES))
    in_maps = []
    for c in core_ids:
        b, g = c // 4, c % 4
        in_maps.append(
            {
                "xT": xTb[b],
                "wq": np.ascontiguousarray(Wq[QSH * g : QSH * (g + 1)].T).astype(bf16),
                "wk": np.ascontiguousarray(Wk[HD * g : HD * (g + 1)].T).astype(bf16),
                "wv": np.ascontiguousarray(Wv[HD * g : HD * (g + 1)].T).astype(bf16),
                "wo": np.ascontiguousarray(Wo[:, QSH * g : QSH * (g + 1)].T).astype(
                    bf16
                ),
                "cosd": cosd,
                "sind": sind,
                "trid": trid,
                "identd": identd,
                "onesd": onesd,
            }
        )
    global LAST_RESULTS
    res = run_bass_kernel_spmd(_get_nc(), in_maps, core_ids, trace=TRACE)
    LAST_RESULTS = res
    out = np.empty((B, T, C), np.float32)
    for b in range(B):
        tot = res.results[4 * b]["out"].astype(np.float32)
        for g in range(1, 4):
            tot = tot + res.results[4 * b + g]["out"].astype(np.float32)
        out[b] = tot
    return out
